# revision 22
# baseline (speedup 1.0000x reference)
"""Bahdanau attention kernel for Trainium2, data-parallel over 8 NeuronCores.

reference (per batch b):
    enc_map = encoder_out[b] @ We + be        # [L, A]
    dec_map = decoder_hidden[b] @ Wd + bd     # [A]
    scores  = tanh(enc_map + dec_map) @ wa + ba   # [L]
    alphas  = softmax(scores)                 # [L]
    context = alphas @ encoder_out[b]         # [ENC]

Sharding: batch 128 -> 16 per core x 8 cores; weights replicated.

Kernel strategy per core:
  - encoder rows loaded as bf16 (gpsimd cast-DMA) in two natural tiles
    [128, 2048] (l=0:128) and [80, 2048] (l=116:196, 12-row overlap so the
    xbar-transpose partition dim is a multiple of 16).
  - xbar DMA-transpose -> TA[e_part, e_chunk, l] for the big matmul
    (contraction over ENC must sit on partitions).
  - enc_mapT[a_part, l] = We_chunk.T @ TA accumulated over 16 e-chunks in
    PSUM (bf16 MACs, fp32 accumulation).
  - ACT fuses (+dec_map bias per-partition, tanh, cast to bf16).
  - scores col [l, 1] = tanh_chunk.T @ wa  (PE, contraction over A).
  - softmax without max subtraction (|scores| <= sum|wa| ~ 22.6, exp is
    safe in fp32); ba dropped entirely (softmax shift invariance).
  - context = (e @ enc) * (1/sum e) with e as the stationary operand.
  - Outputs within ~4e-3 max-rel of the fp32 reference (bf16 rounding).
"""

import numpy as np

B, L, ENC, DEC, ATT = 128, 196, 2048, 512, 512
NCORES = 8
BPC = B // NCORES            # batches per core
EC = ENC // 128              # 16 e-chunks
AC = ATT // 128              # 4 a-chunks
DC = DEC // 128              # 4 d-chunks
LO1 = L - 80                 # 116: start row of second natural chunk
# ctx/scores contraction chunks: (start, size, which natural tile, row offset)
CK = ((0, 116), (116, 80))


def build_nc(loop_iters=None, debug=False):
    import concourse.tile as tile
    from concourse import bacc, mybir

    F32 = mybir.dt.float32
    BF16 = mybir.dt.bfloat16
    TANH = mybir.ActivationFunctionType.Tanh
    EXP = mybir.ActivationFunctionType.Exp

    nc = bacc.Bacc("TRN2", target_bir_lowering=False, debug=False,
                   num_devices=NCORES)
    enc_d = nc.dram_tensor("enc", [BPC, L, ENC], F32, kind="ExternalInput").ap()
    dec_d = nc.dram_tensor("dec", [BPC, DEC], F32, kind="ExternalInput").ap()
    We_d = nc.dram_tensor("We", [ENC, ATT], F32, kind="ExternalInput").ap()
    be_d = nc.dram_tensor("be", [ATT], F32, kind="ExternalInput").ap()
    Wd_d = nc.dram_tensor("Wd", [DEC, ATT], F32, kind="ExternalInput").ap()
    bd_d = nc.dram_tensor("bd", [ATT], F32, kind="ExternalInput").ap()
    wa_d = nc.dram_tensor("wa", [ATT], F32, kind="ExternalInput").ap()
    ctx_d = nc.dram_tensor("context", [BPC, ENC], F32, kind="ExternalOutput").ap()
    alp_d = nc.dram_tensor("alphas", [BPC, L], F32, kind="ExternalOutput").ap()
    if debug:
        dbg_ta = nc.dram_tensor("dbg_ta", [128, EC * 128], BF16, kind="ExternalOutput").ap()
        dbg_nat = nc.dram_tensor("dbg_nat", [128, ENC], BF16, kind="ExternalOutput").ap()
        dbg_tanh = nc.dram_tensor("dbg_tanh", [128, AC * L], BF16, kind="ExternalOutput").ap()
        dbg_s = nc.dram_tensor("dbg_s", [128, 4], F32, kind="ExternalOutput").ap()

    with tile.TileContext(nc) as tc:
        with (
            tc.tile_pool(name="const", bufs=1) as constp,
            tc.tile_pool(name="natb", bufs=5) as natp,
            tc.tile_pool(name="ta", bufs=3) as tap,
            tc.tile_pool(name="tanh", bufs=3) as tanhp,
            tc.tile_pool(name="misc", bufs=3) as miscp,
            tc.tile_pool(name="encps", bufs=4, space="PSUM") as encps,
            tc.tile_pool(name="ctxps", bufs=1, space="PSUM") as ctxps,
            tc.tile_pool(name="smallps", bufs=2, space="PSUM") as smallps,
        ):
            # ---------------- constants / weights ----------------
            # Few big cast-DMAs (SWDGE emission is ~1.3us each, serial on the
            # Pool queue) and batch-0 loads emitted right behind We so the
            # main pipeline starts filling immediately.
            We_bf = [constp.tile([128, 4, ATT], BF16, tag=f"We{j}", name=f"We{j}")
                     for j in range(4)]
            Wd_bf = constp.tile([128, DC, ATT], BF16)
            dec_bf = constp.tile([BPC, DEC], BF16)
            wa_bf = constp.tile([128, AC], BF16)
            # We/Wd ride the idle ACT HWDGE queue as fp32 and are cast to
            # bf16 on DVE, keeping the SWDGE (Pool) queue free for the
            # encoder cast-loads that gate the pipeline start.
            with tc.tile_pool(name="stag", bufs=1) as stagp:
                wdstage = stagp.tile([128, DC, ATT], F32, tag="stw", name="wdstage")
                nc.scalar.dma_start(
                    wdstage[:], Wd_d.rearrange("(c p) a -> p c a", p=128))
                nc.vector.tensor_copy(Wd_bf[:], wdstage[:])
                for j in range(4):
                    west = stagp.tile([128, 4, ATT], F32, tag=f"st{j}", name=f"west{j}")
                    nc.scalar.dma_start(
                        west[:],
                        We_d[j * 512:(j + 1) * 512, :].rearrange(
                            "(c p) a -> p c a", p=128))
                    nc.vector.tensor_copy(We_bf[j][:], west[:])

            decT = [constp.tile([128, BPC], BF16, tag=f"decT{dc}", name=f"decT{dc}")
                    for dc in range(DC)]
            be_sb = constp.tile([128, AC], F32)
            bd_sb = constp.tile([128, AC], F32)
            bias_a = constp.tile([128, AC], F32)
            # 128-wide ones: S-matmul with this as stationary replicates the
            # softmax denominator across all 128 output partitions, so the
            # reciprocal is directly usable as a per-partition scalar.
            ones_bf = constp.tile([128, 128], BF16)
            dec_map = constp.tile([128, AC, BPC], F32)

            # ---------------- main loop ----------------
            # Software-pipelined: batch b's loads/transposes/enc-matmuls are
            # interleaved with batch b-1's tail (scores/softmax/context) so PE
            # never waits on the ACT/DVE softmax chain.
            state = {}

            def loads(b):
                natB0 = natp.tile([128, ENC], BF16, tag="nat0")
                natB1 = natp.tile([80, ENC], BF16, tag="nat1")
                nc.gpsimd.dma_start(natB0[:], enc_d[b, 0:128, :])
                nc.gpsimd.dma_start(natB1[:], enc_d[b, LO1:L, :])
                # xbar transpose: out[p, ec, m] = in[m, ec*128 + p]; the out AP
                # must be a whole contiguous tile (sliced outs misplace data)
                TA0 = tap.tile([128, EC, 128], BF16, tag="ta0")
                TA1 = tap.tile([128, EC, 80], BF16, tag="ta1")
                nc.sync.dma_start(TA0[:], natB0[:], transpose=True)
                nc.sync.dma_start(TA1[:], natB1[:], transpose=True)
                tanh_sb = tanhp.tile([128, AC, L], BF16)
                st = {"nat": (natB0, natB1), "ta": (TA0, TA1), "tanh": tanh_sb}
                state[b] = st
                if debug and b == 0:
                    nc.scalar.dma_start(dbg_ta[:], TA0[:])
                    nc.scalar.dma_start(dbg_nat[:], natB0[:])

            def enc_group(b, ac):
                TA0, TA1 = state[b]["ta"]
                tanh_sb = state[b]["tanh"]
                a0 = ac * 128
                eps = encps.tile([128, L], F32, tag="eps")
                # one accumulation group per bank: start=True clears
                # has_written for the WHOLE bank, so only the very first
                # matmul may carry it; start=False writes still overwrite
                # elements whose bit is unset (per-element semantics).
                for ec in range(EC):
                    wsl = We_bf[ec // 4][:, ec % 4, a0:a0 + 128]
                    nc.tensor.matmul(eps[:, 0:LO1], wsl,
                                     TA0[:, ec, 0:LO1],
                                     start=(ec == 0), stop=False,
                                     skip_group_check=True)
                    nc.tensor.matmul(eps[:, LO1:L], wsl,
                                     TA1[:, ec, :],
                                     start=False, stop=(ec == EC - 1),
                                     skip_group_check=True)
                nc.scalar.activation(tanh_sb[:, ac, :], eps[:], TANH,
                                     bias=dec_map[:, ac, b:b + 1])
                if debug and b == 0 and ac == AC - 1:
                    nc.scalar.dma_start(dbg_tanh[:], tanh_sb[:])

            def t_scores(b):
                st = state[b]
                sps = smallps.tile([128, 4], F32)
                st["sps"] = sps
                st["ecols"] = []
                tanh_sb = st["tanh"]
                for i, (l0, lsz) in enumerate(CK):
                    sout = sps[0:lsz, i:i + 1]
                    for ac in range(AC):
                        nc.tensor.matmul(sout, tanh_sb[:, ac, l0:l0 + lsz],
                                         wa_bf[:, ac:ac + 1],
                                         start=(ac == 0), stop=(ac == AC - 1))
                    ecol = miscp.tile([lsz, 1], BF16, tag=f"e{i}", name=f"ecol{i}")
                    nc.scalar.activation(ecol[:], sout, EXP)
                    st["ecols"].append(ecol)

            def t_S(b):
                st = state[b]
                sps, ecols = st["sps"], st["ecols"]
                # S replicated on all 128 partitions via 128-wide ones lhsT
                Sps = sps[:, 2:3]
                nc.tensor.matmul(Sps, ones_bf[0:116, :], ecols[0][:],
                                 start=True, stop=False)
                nc.tensor.matmul(Sps, ones_bf[0:80, :], ecols[1][:],
                                 start=False, stop=True)
                recip = miscp.tile([128, 1], F32, tag="recip")
                nc.vector.reciprocal(recip[:], Sps)
                st["recip"] = recip
                if debug and b == 0:
                    dbg_sb = miscp.tile([128, 4], F32, tag="dbgs")
                    nc.vector.tensor_copy(dbg_sb[:], sps[:])
                    nc.scalar.dma_start(dbg_s[:], dbg_sb[:])
                # alphas out
                for i, (l0, lsz) in enumerate(CK):
                    acol = miscp.tile([128, 1], F32, tag="acol", name=f"acol{i}")
                    nc.vector.tensor_scalar_mul(acol[0:lsz, :], ecols[i][:],
                                                recip[0:lsz, :])
                    nc.sync.dma_start(alp_d[b, l0:l0 + lsz], acol[0:lsz, :])

            def t_ctx(b, half):
                st = state[b]
                natB0, natB1 = st["nat"]
                ecols, recip = st["ecols"], st["recip"]
                if half == 0:
                    st["csb"] = miscp.tile([1, ENC], F32, tag="csb", name="csb")
                csb = st["csb"]
                cps = ctxps.tile([1, 2, 512], F32)
                for j in range(2):
                    n0 = (half * 2 + j) * 512
                    nc.tensor.matmul(cps[:, j, :], ecols[0][:],
                                     natB0[0:116, n0:n0 + 512],
                                     start=True, stop=False)
                    nc.tensor.matmul(cps[:, j, :], ecols[1][:],
                                     natB1[0:80, n0:n0 + 512],
                                     start=False, stop=True)
                    nc.vector.tensor_scalar_mul(csb[:, n0:n0 + 512],
                                                cps[:, j, :], recip[0:1, :])
                if half == 1:
                    nc.sync.dma_start(ctx_d[b, :], csb[:])
                    del state[b]

            def const_tail():
                nc.gpsimd.dma_start(dec_bf[:], dec_d[:])
                nc.gpsimd.dma_start(wa_bf[:], wa_d.rearrange("(c p) -> p c", p=128))
                for dc in range(DC):
                    nc.sync.dma_start(decT[dc][:],
                                      dec_bf[:, dc * 128:(dc + 1) * 128],
                                      transpose=True)
                nc.scalar.dma_start(be_sb[:], be_d.rearrange("(c p) -> p c", p=128))
                nc.scalar.dma_start(bd_sb[:], bd_d.rearrange("(c p) -> p c", p=128))
                nc.vector.tensor_add(bias_a[:], be_sb[:], bd_sb[:])
                nc.vector.memset(ones_bf[:], 1.0)
                for ac in range(AC):
                    dmps = encps.tile([128, L], F32, tag="eps")
                    out = dmps[:, 0:BPC]
                    for dc in range(DC):
                        nc.tensor.matmul(out,
                                         Wd_bf[:, dc, ac * 128:(ac + 1) * 128],
                                         decT[dc][:],
                                         start=(dc == 0), stop=(dc == DC - 1))
                    nc.vector.tensor_scalar_add(dec_map[:, ac, :], out,
                                                bias_a[:, ac:ac + 1])

            def body():
                # 2-deep software pipeline: batch b's enc groups interleave
                # with batch b-1's scores/S and batch b-2's context, so every
                # cross-engine dependency is at least one enc group old when
                # PE reaches it. Loads run two batches ahead.
                loads(0)
                const_tail()
                loads(1)
                for b in range(BPC):
                    enc_group(b, 0)
                    if b >= 2:
                        t_ctx(b - 2, 0)
                    enc_group(b, 1)
                    if b >= 2:
                        t_ctx(b - 2, 1)
                    if b + 2 < BPC:
                        loads(b + 2)
                    enc_group(b, 2)
                    if b >= 1:
                        t_scores(b - 1)
                    enc_group(b, 3)
                    if b >= 1:
                        t_S(b - 1)
                bl = BPC - 1
                t_ctx(bl - 1, 0); t_ctx(bl - 1, 1)
                t_scores(bl); t_S(bl); t_ctx(bl, 0); t_ctx(bl, 1)

            if loop_iters is None:
                body()
            else:
                with tc.For_i(0, loop_iters, 1):
                    body()

    nc.compile()
    return nc


_cached = {}


def _get_nc(key=("plain", None)):
    if key not in _cached:
        kind, iters = key
        _cached[key] = build_nc(loop_iters=iters, debug=(kind == "debug"))
    return _cached[key]


def run_sharded(nc, encoder_out, decoder_hidden, We, be, Wd, bd, wa, **kw):
    from concourse.bass_utils import run_bass_kernel_spmd

    encoder_out = np.ascontiguousarray(np.asarray(encoder_out, dtype=np.float32))
    decoder_hidden = np.ascontiguousarray(np.asarray(decoder_hidden, dtype=np.float32))
    shared = {
        "We": np.ascontiguousarray(np.asarray(We, dtype=np.float32)),
        "be": np.ascontiguousarray(np.asarray(be, dtype=np.float32)),
        "Wd": np.ascontiguousarray(np.asarray(Wd, dtype=np.float32)),
        "bd": np.ascontiguousarray(np.asarray(bd, dtype=np.float32)),
        "wa": np.ascontiguousarray(np.asarray(wa, dtype=np.float32)),
    }
    in_maps = []
    for c in range(NCORES):
        sl = slice(c * BPC, (c + 1) * BPC)
        in_maps.append({"enc": encoder_out[sl], "dec": decoder_hidden[sl], **shared})
    res = run_bass_kernel_spmd(nc, in_maps, list(range(NCORES)), **kw)
    context = np.concatenate([res.results[c]["context"] for c in range(NCORES)], axis=0)
    alphas = np.concatenate([res.results[c]["alphas"] for c in range(NCORES)], axis=0)
    return (context, alphas), res


def kernel(encoder_out, decoder_hidden, We, be, Wd, bd, wa, ba):
    # ba is dropped: alphas = softmax(scores + ba) == softmax(scores), and
    # neither output depends on it otherwise.
    del ba
    nc = _get_nc()
    (context, alphas), _ = run_sharded(nc, encoder_out, decoder_hidden,
                                       We, be, Wd, bd, wa)
    return (context, alphas)
